# revision 21
# baseline (speedup 1.0000x reference)
"""Trainium2 Bass kernel for the masked multi-head attention module.

Shapes (hardcoded): B=4, SQ=SK=1024, D=1024, H=16, DH=64.
Sharding over 8 cores: core c -> batch b=c//2, head-half hh=c%2 (8 heads),
output-column-half hh. Pairwise AllGather of ctx^T between cores (2b, 2b+1),
then each core computes a disjoint 512-column slice of the output.

All matmuls run as float32r (FP22 truncated fp32, full PE rate). Scores are
computed transposed (S^T[k, q]) so the masked softmax exp fuses mask bias and
PSUM->SBUF eviction into one ScalarE activation, and the softmax denominators
come out of the ctx matmul for free via a ones-column appended to V.
"""

import os
import numpy as np

B, S, D, H, DH = 4, 1024, 1024, 16, 64
P = 128
NEG = -1.0e9

_CACHE = {}
LAST_RESULT = None


def _build_program():
    from concourse import bacc
    import concourse.bass as bass
    import concourse.tile as tile
    from concourse import mybir

    f32 = mybir.dt.float32
    f32r = mybir.dt.float32r
    Exp = mybir.ActivationFunctionType.Exp

    nc = bacc.Bacc("TRN2", target_bir_lowering=False, debug=False, num_devices=8)

    qT_in = nc.dram_tensor("qT_in", [D, S], f32, kind="ExternalInput")
    vT_in = nc.dram_tensor("vT_in", [D, S], f32, kind="ExternalInput")
    wq_d = nc.dram_tensor("wq", [D, 512], f32, kind="ExternalInput")
    wk_d = nc.dram_tensor("wk", [D, 512], f32, kind="ExternalInput")
    wv_d = nc.dram_tensor("wv", [D, 512], f32, kind="ExternalInput")
    wo_d = nc.dram_tensor("wo", [H * DH, 512], f32, kind="ExternalInput")
    bq_d = nc.dram_tensor("bq2", [P, 4], f32, kind="ExternalInput")
    bk_d = nc.dram_tensor("bk2", [P, 4], f32, kind="ExternalInput")
    bv_d = nc.dram_tensor("bv_row", [1, 512], f32, kind="ExternalInput")
    bo_d = nc.dram_tensor("bo_row", [1, 512], f32, kind="ExternalInput")
    vb_d = nc.dram_tensor("vbias", [P, 8], f32, kind="ExternalInput")
    qm_d = nc.dram_tensor("qm_rsh", [P, 16], f32, kind="ExternalInput")
    y_out = nc.dram_tensor("y_out", [S, 512], f32, kind="ExternalOutput")

    groups = [[0, 1], [2, 3], [4, 5], [6, 7]]

    def bcast_ap(src_ap, nparts):
        # partition-broadcast read (stride-0 partition dim); DRAM source only
        return bass.AP(
            tensor=src_ap.tensor,
            offset=src_ap.offset,
            ap=[[0, nparts]] + list(src_ap.ap[1:]),
        )

    with tile.TileContext(nc) as tc:
        with (
            tc.tile_pool(name="A", bufs=30) as A,
            tc.tile_pool(name="W", bufs=26) as Wp,
            tc.tile_pool(name="VS", bufs=8) as VSp,
            tc.tile_pool(name="SM", bufs=1) as SM,
            tc.tile_pool(name="SM2", bufs=1) as SM2,
            tc.tile_pool(name="ps_sc", bufs=2, space="PSUM") as PSC,
            tc.tile_pool(name="ps_wk", bufs=2, space="PSUM") as PSW,
            tc.tile_pool(name="ps_cx", bufs=2, space="PSUM") as PSX,
            tc.tile_pool(name="dram", bufs=4, space="DRAM") as DR,
        ):
            # ---- small constants ----
            bq_sb = SM.tile([P, 4], f32, tag="bq")
            nc.sync.dma_start(out=bq_sb[:], in_=bq_d[:, :])
            bk_sb = SM.tile([P, 4], f32, tag="bk")
            nc.sync.dma_start(out=bk_sb[:], in_=bk_d[:, :])
            vb_sb = SM.tile([P, 8], f32, tag="vb")
            nc.sync.dma_start(out=vb_sb[:], in_=vb_d[:, :])
            qm_sb = SM.tile([P, 16], f32, tag="qm")
            nc.sync.dma_start(out=qm_sb[:], in_=qm_d[:, :])
            bv_bc = SM.tile([P, 512], f32, tag="bvb")
            nc.gpsimd.dma_start(out=bv_bc[:], in_=bcast_ap(bv_d[:, :], P))
            bo_bc = SM.tile([P, 512], f32, tag="bob")
            nc.gpsimd.dma_start(out=bo_bc[:], in_=bcast_ap(bo_d[:, :], P))

            # ---- big loads (q/v pre-transposed on host) ----
            def load_xt(x_dram):
                out = []
                for i in range(8):
                    t = A.tile([P, S], f32, tag="big")
                    nc.sync.dma_start(
                        out=t[:].bitcast(f32r),
                        in_=x_dram[i * P:(i + 1) * P, :].bitcast(f32r),
                    )
                    out.append(t)
                return out

            def load_w(w_dram):
                out = []
                for i in range(8):
                    t = Wp.tile([P, 512], f32, tag="w")
                    nc.sync.dma_start(
                        out=t[:].bitcast(f32r),
                        in_=w_dram[i * P:(i + 1) * P, :].bitcast(f32r),
                    )
                    out.append(t)
                return out

            qT = load_xt(qT_in)
            wq_sb = load_w(wq_d)
            vT = load_xt(vT_in)
            wk_sb = load_w(wk_d)
            wv_sb = load_w(wv_d)

            QTp = [None] * 8   # per-head zero-padded Q^T [128, S]
            KT = [None] * 4    # stacked K^T head pairs [128, S]
            Vst = [None] * 8   # V with ones column [128, 8, 65]

            def q_group(ht):
                # Q^T for head pair ht -> two zero-padded per-head tiles
                tA = A.tile([P, S], f32, tag="big")
                nc.vector.memset(tA[64:128, :], 0.0)
                tB = A.tile([P, S], f32, tag="big")
                nc.vector.memset(tB[0:64, :], 0.0)
                for c in range(2):
                    cs = slice(c * 512, (c + 1) * 512)
                    ps = PSW.tile([P, 512], f32, tag="work")
                    for di in range(8):
                        nc.tensor.matmul(
                            ps[:, :],
                            lhsT=wq_sb[di][:, ht * P:(ht + 1) * P].bitcast(f32r),
                            rhs=qT[di][:, cs].bitcast(f32r),
                            start=(di == 0),
                            stop=(di == 7),
                        )
                    nc.vector.tensor_scalar_add(
                        tA[0:64, cs].bitcast(f32r), ps[0:64, :], bq_sb[0:64, ht:ht + 1]
                    )
                    nc.vector.tensor_scalar_add(
                        tB[64:128, cs].bitcast(f32r), ps[64:128, :], bq_sb[64:128, ht:ht + 1]
                    )
                QTp[2 * ht], QTp[2 * ht + 1] = tA, tB

            def k_group(ht):
                t = A.tile([P, S], f32, tag="big")
                for c in range(2):
                    cs = slice(c * 512, (c + 1) * 512)
                    ps = PSW.tile([P, 512], f32, tag="work")
                    for di in range(8):
                        nc.tensor.matmul(
                            ps[:, :],
                            lhsT=wk_sb[di][:, ht * P:(ht + 1) * P].bitcast(f32r),
                            rhs=vT[di][:, cs].bitcast(f32r),
                            start=(di == 0),
                            stop=(di == 7),
                        )
                    nc.vector.tensor_scalar_add(
                        t[:, cs].bitcast(f32r), ps[:, :], bk_sb[:, ht:ht + 1]
                    )
                KT[ht] = t

            def v_group(kt):
                ps = PSW.tile([P, 512], f32, tag="work")
                for di in range(8):
                    nc.tensor.matmul(
                        ps[:, :],
                        lhsT=vT[di][:, kt * P:(kt + 1) * P].bitcast(f32r),
                        rhs=wv_sb[di][:, :].bitcast(f32r),
                        start=(di == 0),
                        stop=(di == 7),
                    )
                t = VSp.tile([P, 8, 65], f32, tag="vst")
                nc.vector.memset(t[:], 1.0)
                nc.vector.tensor_add(
                    t[:, :, 0:64].bitcast(f32r),
                    ps[:, :].rearrange("p (h d) -> p h d", h=8),
                    bv_bc[:, :].rearrange("p (h d) -> p h d", h=8),
                )
                Vst[kt] = t

            ctxT_full = [None] * 8

            def pair(p):
                st = A.tile([P, S], f32, tag="big")
                sumA = SM2.tile([1, S], f32, tag="sumA")
                sumB = SM2.tile([1, S], f32, tag="sumB")
                for c in range(2):
                    cs = slice(c * 512, (c + 1) * 512)
                    ctxA = PSX.tile([65, 512], f32, tag="ctx")
                    ctxB = PSX.tile([65, 512], f32, tag="ctx")
                    for kt in range(8):
                        sps = PSC.tile([P, S], f32, tag="sc")
                        nc.tensor.matmul(
                            sps[:, 0:512],
                            lhsT=KT[p][:, kt * P:(kt + 1) * P].bitcast(f32r),
                            rhs=QTp[2 * p][:, cs].bitcast(f32r),
                            start=True,
                            stop=True,
                        )
                        nc.tensor.matmul(
                            sps[:, 512:1024],
                            lhsT=KT[p][:, kt * P:(kt + 1) * P].bitcast(f32r),
                            rhs=QTp[2 * p + 1][:, cs].bitcast(f32r),
                            start=True,
                            stop=True,
                        )
                        ut = A.tile([P, S], f32, tag="big")
                        nc.scalar.activation(
                            ut[:].bitcast(f32r), sps[:], Exp,
                            bias=vb_sb[:, kt:kt + 1], scale=1.0,
                        )
                        nc.tensor.matmul(
                            ctxA[:, :],
                            lhsT=Vst[kt][:, 2 * p, :].bitcast(f32r),
                            rhs=ut[:, 0:512].bitcast(f32r),
                            start=(kt == 0),
                            stop=(kt == 7),
                        )
                        nc.tensor.matmul(
                            ctxB[:, :],
                            lhsT=Vst[kt][:, 2 * p + 1, :].bitcast(f32r),
                            rhs=ut[:, 512:1024].bitcast(f32r),
                            start=(kt == 0),
                            stop=(kt == 7),
                        )
                    # evict ctx + sums out of PSUM right away (frees psum for
                    # the next chunk/pair); normalization happens on the copy
                    nc.vector.tensor_copy(st[0:64, cs], ctxA[0:64, :])
                    nc.vector.tensor_copy(sumA[0:1, cs], ctxA[64:65, :])
                    nc.vector.tensor_copy(st[64:128, cs], ctxB[0:64, :])
                    nc.vector.tensor_copy(sumB[0:1, cs], ctxB[64:65, :])

                # r = q_mask / sums, broadcast over partitions via DRAM
                rsh = SM2.tile([P, 16], f32, tag="rsh")
                nc.gpsimd.dma_start(out=rsh[0:64, :], in_=sumA[:])
                nc.gpsimd.dma_start(out=rsh[64:128, :], in_=sumB[:])
                rr = SM2.tile([P, 16], f32, tag="rr")
                nc.vector.reciprocal(rr[:], rsh[:])
                nc.vector.tensor_mul(rr[:], rr[:], qm_sb[:])
                rdram = DR.tile([2, S], f32, tag="rd")
                nc.gpsimd.dma_start(out=rdram[0:1, :], in_=rr[0:64, :])
                nc.gpsimd.dma_start(out=rdram[1:2, :], in_=rr[64:128, :])
                bc = A.tile([P, S], f32, tag="big")
                nc.gpsimd.dma_start(out=bc[0:64, :], in_=bcast_ap(rdram[0:1, :], 64))
                nc.gpsimd.dma_start(out=bc[64:128, :], in_=bcast_ap(rdram[1:2, :], 64))
                nc.vector.tensor_mul(st[:].bitcast(f32r), st[:], bc[:])

                # pairwise exchange of ctx^T
                cin = DR.tile([P, S], f32, tag="ccin")
                nc.gpsimd.dma_start(out=cin[:], in_=st[:])
                cout = DR.tile([2, P, S], f32, tag="ccout")
                nc.gpsimd.collective_compute(
                    "AllGather",
                    mybir.AluOpType.bypass,
                    replica_groups=groups,
                    ins=[cin[:].opt()],
                    outs=[cout[:].opt()],
                )
                ta = A.tile([P, S], f32, tag="big")
                nc.gpsimd.dma_start(out=ta[:].bitcast(f32r), in_=cout[0, :, :].bitcast(f32r))
                tb = A.tile([P, S], f32, tag="big")
                nc.gpsimd.dma_start(out=tb[:].bitcast(f32r), in_=cout[1, :, :].bitcast(f32r))
                ctxT_full[p] = ta
                ctxT_full[4 + p] = tb

            # ---- emit: projections upfront (frees qT/vT early), then pairs
            q_group(0)
            k_group(0)
            q_group(1)
            k_group(1)
            q_group(2)
            k_group(2)
            q_group(3)
            k_group(3)
            for kt in range(8):
                v_group(kt)
            wo_sb = load_w(wo_d)
            for p in range(4):
                pair(p)

            # ---- output projection (column slice), gather-arrival order ----
            HT_ORDER = [0, 4, 1, 5, 2, 6, 3, 7]
            for qt in range(8):
                yp = PSW.tile([P, 512], f32, tag="work")
                for i, ht in enumerate(HT_ORDER):
                    nc.tensor.matmul(
                        yp[:, 0:512],
                        lhsT=ctxT_full[ht][:, qt * P:(qt + 1) * P].bitcast(f32r),
                        rhs=wo_sb[ht][:, :].bitcast(f32r),
                        start=(i == 0),
                        stop=(i == 7),
                    )
                ysb = Wp.tile([P, 512], f32, tag="w")
                nc.vector.tensor_add(ysb[:], yp[:, 0:512], bo_bc[:])
                nc.sync.dma_start(out=y_out[qt * P:(qt + 1) * P, :], in_=ysb[:])

    nc.compile()
    return nc


def _get_program():
    if "nc" not in _CACHE:
        _CACHE["nc"] = _build_program()
    return _CACHE["nc"]


def kernel(q, v, q_mask, v_mask, Wq, bq, Wk, bk, Wv, bv, Wo, bo):
    global LAST_RESULT
    from concourse.bass_utils import run_bass_kernel_spmd

    q = np.asarray(q, dtype=np.float32)
    v = np.asarray(v, dtype=np.float32)
    q_mask = np.asarray(q_mask)
    v_mask = np.asarray(v_mask)
    Wq = np.asarray(Wq, dtype=np.float32)
    Wk = np.asarray(Wk, dtype=np.float32)
    Wv = np.asarray(Wv, dtype=np.float32)
    Wo = np.asarray(Wo, dtype=np.float32)
    bq = np.asarray(bq, dtype=np.float32)
    bk = np.asarray(bk, dtype=np.float32)
    bv = np.asarray(bv, dtype=np.float32)
    bo = np.asarray(bo, dtype=np.float32)

    nc = _get_program()

    in_maps = []
    for c in range(8):
        b, hh = c // 2, c % 2
        hsl = slice(512 * hh, 512 * (hh + 1))
        vb = np.where(v_mask[b], 0.0, NEG).astype(np.float32)
        qm = q_mask[b].astype(np.float32)
        in_maps.append(
            {
                "qT_in": np.ascontiguousarray(q[b].T),
                "vT_in": np.ascontiguousarray(v[b].T),
                "wq": np.ascontiguousarray(Wq[:, hsl]),
                "wk": np.ascontiguousarray(Wk[:, hsl]),
                "wv": np.ascontiguousarray(Wv[:, hsl]),
                "wo": np.ascontiguousarray(Wo[:, hsl]),
                "bq2": np.ascontiguousarray(bq[hsl].reshape(4, P).T),
                "bk2": np.ascontiguousarray(bk[hsl].reshape(4, P).T),
                "bv_row": np.ascontiguousarray(bv[hsl].reshape(1, 512)),
                "bo_row": np.ascontiguousarray(bo[hsl].reshape(1, 512)),
                "vbias": np.ascontiguousarray(vb.reshape(8, P).T),
                "qm_rsh": np.ascontiguousarray(
                    np.tile(qm.reshape(64, 16), (2, 1))
                ),
            }
        )

    res = run_bass_kernel_spmd(
        nc,
        in_maps,
        core_ids=list(range(8)),
        tmpdir=os.environ.get("KERNEL_TRACE_DIR") or None,
    )
    LAST_RESULT = res

    out = np.empty((B, S, D), dtype=np.float32)
    for b in range(B):
        out[b, :, 0:512] = res.results[2 * b]["y_out"]
        out[b, :, 512:1024] = res.results[2 * b + 1]["y_out"]
    return out


# revision 22
# speedup vs baseline: 1.0239x; 1.0239x over previous
"""Trainium2 Bass kernel for the masked multi-head attention module.

Shapes (hardcoded): B=4, SQ=SK=1024, D=1024, H=16, DH=64.
Sharding over 8 cores: core c -> batch b=c//2, head-half hh=c%2 (8 heads),
output-column-half hh. Pairwise AllGather of ctx^T between cores (2b, 2b+1),
then each core computes a disjoint 512-column slice of the output.

All matmuls run as float32r (FP22 truncated fp32, full PE rate). Scores are
computed transposed (S^T[k, q]) so the masked softmax exp fuses mask bias and
PSUM->SBUF eviction into one ScalarE activation, and the softmax denominators
come out of the ctx matmul for free via a ones-column appended to V.
"""

import os
import numpy as np

B, S, D, H, DH = 4, 1024, 1024, 16, 64
P = 128
NEG = -1.0e9

_CACHE = {}
LAST_RESULT = None


def _build_program():
    from concourse import bacc
    import concourse.bass as bass
    import concourse.tile as tile
    from concourse import mybir

    f32 = mybir.dt.float32
    f32r = mybir.dt.float32r
    Exp = mybir.ActivationFunctionType.Exp

    nc = bacc.Bacc("TRN2", target_bir_lowering=False, debug=False, num_devices=8)

    qT_in = nc.dram_tensor("qT_in", [D, S], f32, kind="ExternalInput")
    vT_in = nc.dram_tensor("vT_in", [D, S], f32, kind="ExternalInput")
    wq_d = nc.dram_tensor("wq", [D, 512], f32, kind="ExternalInput")
    wk_d = nc.dram_tensor("wk", [D, 512], f32, kind="ExternalInput")
    wv_d = nc.dram_tensor("wv", [D, 512], f32, kind="ExternalInput")
    wo_d = nc.dram_tensor("wo", [H * DH, 512], f32, kind="ExternalInput")
    bq_d = nc.dram_tensor("bq2", [P, 4], f32, kind="ExternalInput")
    bk_d = nc.dram_tensor("bk2", [P, 4], f32, kind="ExternalInput")
    bv_d = nc.dram_tensor("bv_row", [1, 512], f32, kind="ExternalInput")
    bo_d = nc.dram_tensor("bo_row", [1, 512], f32, kind="ExternalInput")
    vb_d = nc.dram_tensor("vbias", [P, 8], f32, kind="ExternalInput")
    qm_d = nc.dram_tensor("qm_rsh", [P, 16], f32, kind="ExternalInput")
    y_out = nc.dram_tensor("y_out", [S, 512], f32, kind="ExternalOutput")

    groups = [[0, 1], [2, 3], [4, 5], [6, 7]]

    def bcast_ap(src_ap, nparts):
        # partition-broadcast read (stride-0 partition dim); DRAM source only
        return bass.AP(
            tensor=src_ap.tensor,
            offset=src_ap.offset,
            ap=[[0, nparts]] + list(src_ap.ap[1:]),
        )

    with tile.TileContext(nc) as tc:
        with (
            tc.tile_pool(name="A", bufs=26) as A,
            tc.tile_pool(name="STBC", bufs=2) as STBC,
            tc.tile_pool(name="W", bufs=26) as Wp,
            tc.tile_pool(name="VS", bufs=8) as VSp,
            tc.tile_pool(name="SM", bufs=1) as SM,
            tc.tile_pool(name="SM2", bufs=1) as SM2,
            tc.tile_pool(name="ps_sc", bufs=2, space="PSUM") as PSC,
            tc.tile_pool(name="ps_wk", bufs=2, space="PSUM") as PSW,
            tc.tile_pool(name="ps_cx", bufs=2, space="PSUM") as PSX,
            tc.tile_pool(name="dram", bufs=4, space="DRAM") as DR,
        ):
            # ---- small constants ----
            bq_sb = SM.tile([P, 4], f32, tag="bq")
            nc.sync.dma_start(out=bq_sb[:], in_=bq_d[:, :])
            bk_sb = SM.tile([P, 4], f32, tag="bk")
            nc.sync.dma_start(out=bk_sb[:], in_=bk_d[:, :])
            vb_sb = SM.tile([P, 8], f32, tag="vb")
            nc.sync.dma_start(out=vb_sb[:], in_=vb_d[:, :])
            qm_sb = SM.tile([P, 16], f32, tag="qm")
            nc.sync.dma_start(out=qm_sb[:], in_=qm_d[:, :])
            bv_bc = SM.tile([P, 512], f32, tag="bvb")
            nc.gpsimd.dma_start(out=bv_bc[:], in_=bcast_ap(bv_d[:, :], P))
            bo_bc = SM.tile([P, 512], f32, tag="bob")
            nc.gpsimd.dma_start(out=bo_bc[:], in_=bcast_ap(bo_d[:, :], P))

            # ---- big loads (q/v pre-transposed on host) ----
            def load_xt(x_dram):
                out = []
                for i in range(8):
                    t = A.tile([P, S], f32, tag="big")
                    nc.sync.dma_start(
                        out=t[:].bitcast(f32r),
                        in_=x_dram[i * P:(i + 1) * P, :].bitcast(f32r),
                    )
                    out.append(t)
                return out

            def load_w(w_dram):
                out = []
                for i in range(8):
                    t = Wp.tile([P, 512], f32, tag="w")
                    nc.sync.dma_start(
                        out=t[:].bitcast(f32r),
                        in_=w_dram[i * P:(i + 1) * P, :].bitcast(f32r),
                    )
                    out.append(t)
                return out

            qT = load_xt(qT_in)
            wq_sb = load_w(wq_d)
            vT = load_xt(vT_in)
            wk_sb = load_w(wk_d)
            wv_sb = load_w(wv_d)

            QTp = [None] * 8   # per-head zero-padded Q^T [128, S]
            KT = [None] * 4    # stacked K^T head pairs [128, S]
            Vst = [None] * 8   # V with ones column [128, 8, 65]

            def q_group(ht):
                # Q^T for head pair ht -> two zero-padded per-head tiles
                tA = A.tile([P, S], f32, tag="big")
                nc.vector.memset(tA[64:128, :], 0.0)
                tB = A.tile([P, S], f32, tag="big")
                nc.vector.memset(tB[0:64, :], 0.0)
                for c in range(2):
                    cs = slice(c * 512, (c + 1) * 512)
                    ps = PSW.tile([P, 512], f32, tag="work")
                    for di in range(8):
                        nc.tensor.matmul(
                            ps[:, :],
                            lhsT=wq_sb[di][:, ht * P:(ht + 1) * P].bitcast(f32r),
                            rhs=qT[di][:, cs].bitcast(f32r),
                            start=(di == 0),
                            stop=(di == 7),
                        )
                    nc.vector.tensor_scalar_add(
                        tA[0:64, cs].bitcast(f32r), ps[0:64, :], bq_sb[0:64, ht:ht + 1]
                    )
                    nc.vector.tensor_scalar_add(
                        tB[64:128, cs].bitcast(f32r), ps[64:128, :], bq_sb[64:128, ht:ht + 1]
                    )
                QTp[2 * ht], QTp[2 * ht + 1] = tA, tB

            def k_group(ht):
                t = A.tile([P, S], f32, tag="big")
                for c in range(2):
                    cs = slice(c * 512, (c + 1) * 512)
                    ps = PSW.tile([P, 512], f32, tag="work")
                    for di in range(8):
                        nc.tensor.matmul(
                            ps[:, :],
                            lhsT=wk_sb[di][:, ht * P:(ht + 1) * P].bitcast(f32r),
                            rhs=vT[di][:, cs].bitcast(f32r),
                            start=(di == 0),
                            stop=(di == 7),
                        )
                    nc.vector.tensor_scalar_add(
                        t[:, cs].bitcast(f32r), ps[:, :], bk_sb[:, ht:ht + 1]
                    )
                KT[ht] = t

            def v_group(kt):
                ps = PSW.tile([P, 512], f32, tag="work")
                for di in range(8):
                    nc.tensor.matmul(
                        ps[:, :],
                        lhsT=vT[di][:, kt * P:(kt + 1) * P].bitcast(f32r),
                        rhs=wv_sb[di][:, :].bitcast(f32r),
                        start=(di == 0),
                        stop=(di == 7),
                    )
                t = VSp.tile([P, 8, 65], f32, tag="vst")
                nc.vector.memset(t[:], 1.0)
                nc.vector.tensor_add(
                    t[:, :, 0:64].bitcast(f32r),
                    ps[:, :].rearrange("p (h d) -> p h d", h=8),
                    bv_bc[:, :].rearrange("p (h d) -> p h d", h=8),
                )
                Vst[kt] = t

            ctxT_full = [None] * 8

            def pair(p):
                st = STBC.tile([P, S], f32, tag="st")
                sumA = SM2.tile([1, S], f32, tag="sumA")
                sumB = SM2.tile([1, S], f32, tag="sumB")
                for c in range(2):
                    cs = slice(c * 512, (c + 1) * 512)
                    ctxA = PSX.tile([65, 512], f32, tag="ctx")
                    ctxB = PSX.tile([65, 512], f32, tag="ctx")
                    for kt in range(8):
                        sps = PSC.tile([P, S], f32, tag="sc")
                        nc.tensor.matmul(
                            sps[:, 0:512],
                            lhsT=KT[p][:, kt * P:(kt + 1) * P].bitcast(f32r),
                            rhs=QTp[2 * p][:, cs].bitcast(f32r),
                            start=True,
                            stop=True,
                        )
                        nc.tensor.matmul(
                            sps[:, 512:1024],
                            lhsT=KT[p][:, kt * P:(kt + 1) * P].bitcast(f32r),
                            rhs=QTp[2 * p + 1][:, cs].bitcast(f32r),
                            start=True,
                            stop=True,
                        )
                        ut = A.tile([P, S], f32, tag="big")
                        nc.scalar.activation(
                            ut[:].bitcast(f32r), sps[:], Exp,
                            bias=vb_sb[:, kt:kt + 1], scale=1.0,
                        )
                        nc.tensor.matmul(
                            ctxA[:, :],
                            lhsT=Vst[kt][:, 2 * p, :].bitcast(f32r),
                            rhs=ut[:, 0:512].bitcast(f32r),
                            start=(kt == 0),
                            stop=(kt == 7),
                        )
                        nc.tensor.matmul(
                            ctxB[:, :],
                            lhsT=Vst[kt][:, 2 * p + 1, :].bitcast(f32r),
                            rhs=ut[:, 512:1024].bitcast(f32r),
                            start=(kt == 0),
                            stop=(kt == 7),
                        )
                    # evict ctx + sums out of PSUM right away (frees psum for
                    # the next chunk/pair); normalization happens on the copy
                    nc.vector.tensor_copy(st[0:64, cs], ctxA[0:64, :])
                    nc.vector.tensor_copy(sumA[0:1, cs], ctxA[64:65, :])
                    nc.vector.tensor_copy(st[64:128, cs], ctxB[0:64, :])
                    nc.vector.tensor_copy(sumB[0:1, cs], ctxB[64:65, :])

                # r = q_mask / sums, broadcast over partitions via DRAM
                rsh = SM2.tile([P, 16], f32, tag="rsh")
                nc.gpsimd.dma_start(out=rsh[0:64, :], in_=sumA[:])
                nc.gpsimd.dma_start(out=rsh[64:128, :], in_=sumB[:])
                rr = SM2.tile([P, 16], f32, tag="rr")
                nc.vector.reciprocal(rr[:], rsh[:])
                nc.vector.tensor_mul(rr[:], rr[:], qm_sb[:])
                rdram = DR.tile([2, S], f32, tag="rd")
                nc.gpsimd.dma_start(out=rdram[0:1, :], in_=rr[0:64, :])
                nc.gpsimd.dma_start(out=rdram[1:2, :], in_=rr[64:128, :])
                bc = STBC.tile([P, S], f32, tag="bc")
                nc.gpsimd.dma_start(out=bc[0:64, :], in_=bcast_ap(rdram[0:1, :], 64))
                nc.gpsimd.dma_start(out=bc[64:128, :], in_=bcast_ap(rdram[1:2, :], 64))
                nc.vector.tensor_mul(st[:].bitcast(f32r), st[:], bc[:])

                # pairwise exchange of ctx^T
                cin = DR.tile([P, S], f32, tag="ccin")
                nc.gpsimd.dma_start(out=cin[:], in_=st[:])
                cout = DR.tile([2, P, S], f32, tag="ccout")
                nc.gpsimd.collective_compute(
                    "AllGather",
                    mybir.AluOpType.bypass,
                    replica_groups=groups,
                    ins=[cin[:].opt()],
                    outs=[cout[:].opt()],
                )
                ta = A.tile([P, S], f32, tag="big")
                nc.gpsimd.dma_start(out=ta[:].bitcast(f32r), in_=cout[0, :, :].bitcast(f32r))
                tb = A.tile([P, S], f32, tag="big")
                nc.gpsimd.dma_start(out=tb[:].bitcast(f32r), in_=cout[1, :, :].bitcast(f32r))
                ctxT_full[p] = ta
                ctxT_full[4 + p] = tb

            # ---- emit: projections upfront (frees qT/vT early), then pairs
            q_group(0)
            k_group(0)
            q_group(1)
            k_group(1)
            q_group(2)
            k_group(2)
            q_group(3)
            k_group(3)
            for kt in range(8):
                v_group(kt)
            wo_sb = load_w(wo_d)
            for p in range(4):
                pair(p)

            # ---- output projection (column slice), gather-arrival order ----
            HT_ORDER = [0, 4, 1, 5, 2, 6, 3, 7]
            for qt in range(8):
                yp = PSW.tile([P, 512], f32, tag="work")
                for i, ht in enumerate(HT_ORDER):
                    nc.tensor.matmul(
                        yp[:, 0:512],
                        lhsT=ctxT_full[ht][:, qt * P:(qt + 1) * P].bitcast(f32r),
                        rhs=wo_sb[ht][:, :].bitcast(f32r),
                        start=(i == 0),
                        stop=(i == 7),
                    )
                ysb = Wp.tile([P, 512], f32, tag="w")
                nc.vector.tensor_add(ysb[:], yp[:, 0:512], bo_bc[:])
                nc.sync.dma_start(out=y_out[qt * P:(qt + 1) * P, :], in_=ysb[:])

    nc.compile()
    return nc


def _get_program():
    if "nc" not in _CACHE:
        _CACHE["nc"] = _build_program()
    return _CACHE["nc"]


def kernel(q, v, q_mask, v_mask, Wq, bq, Wk, bk, Wv, bv, Wo, bo):
    global LAST_RESULT
    from concourse.bass_utils import run_bass_kernel_spmd

    q = np.asarray(q, dtype=np.float32)
    v = np.asarray(v, dtype=np.float32)
    q_mask = np.asarray(q_mask)
    v_mask = np.asarray(v_mask)
    Wq = np.asarray(Wq, dtype=np.float32)
    Wk = np.asarray(Wk, dtype=np.float32)
    Wv = np.asarray(Wv, dtype=np.float32)
    Wo = np.asarray(Wo, dtype=np.float32)
    bq = np.asarray(bq, dtype=np.float32)
    bk = np.asarray(bk, dtype=np.float32)
    bv = np.asarray(bv, dtype=np.float32)
    bo = np.asarray(bo, dtype=np.float32)

    nc = _get_program()

    in_maps = []
    for c in range(8):
        b, hh = c // 2, c % 2
        hsl = slice(512 * hh, 512 * (hh + 1))
        vb = np.where(v_mask[b], 0.0, NEG).astype(np.float32)
        qm = q_mask[b].astype(np.float32)
        in_maps.append(
            {
                "qT_in": np.ascontiguousarray(q[b].T),
                "vT_in": np.ascontiguousarray(v[b].T),
                "wq": np.ascontiguousarray(Wq[:, hsl]),
                "wk": np.ascontiguousarray(Wk[:, hsl]),
                "wv": np.ascontiguousarray(Wv[:, hsl]),
                "wo": np.ascontiguousarray(Wo[:, hsl]),
                "bq2": np.ascontiguousarray(bq[hsl].reshape(4, P).T),
                "bk2": np.ascontiguousarray(bk[hsl].reshape(4, P).T),
                "bv_row": np.ascontiguousarray(bv[hsl].reshape(1, 512)),
                "bo_row": np.ascontiguousarray(bo[hsl].reshape(1, 512)),
                "vbias": np.ascontiguousarray(vb.reshape(8, P).T),
                "qm_rsh": np.ascontiguousarray(
                    np.tile(qm.reshape(64, 16), (2, 1))
                ),
            }
        )

    res = run_bass_kernel_spmd(
        nc,
        in_maps,
        core_ids=list(range(8)),
        tmpdir=os.environ.get("KERNEL_TRACE_DIR") or None,
    )
    LAST_RESULT = res

    out = np.empty((B, S, D), dtype=np.float32)
    for b in range(B):
        out[b, :, 0:512] = res.results[2 * b]["y_out"]
        out[b, :, 512:1024] = res.results[2 * b + 1]["y_out"]
    return out


# revision 23
# speedup vs baseline: 1.2856x; 1.2557x over previous
"""Trainium2 Bass kernel for the masked multi-head attention module.

Shapes (hardcoded): B=4, SQ=SK=1024, D=1024, H=16, DH=64.
Sharding over 8 cores: core c -> batch b=c//2, head-half hh=c%2 (8 heads),
output-column-half hh. Pairwise AllGather of ctx^T between cores (2b, 2b+1),
then each core computes a disjoint 512-column slice of the output.

All matmuls run as float32r (FP22 truncated fp32, full PE rate). Scores are
computed transposed (S^T[k, q]) so the masked softmax exp fuses mask bias and
PSUM->SBUF eviction into one ScalarE activation, and the softmax denominators
come out of the ctx matmul for free via a ones-column appended to V.
"""

import os
import numpy as np

B, S, D, H, DH = 4, 1024, 1024, 16, 64
P = 128
NEG = -1.0e9

_CACHE = {}
LAST_RESULT = None


def _build_program():
    from concourse import bacc
    import concourse.bass as bass
    import concourse.tile as tile
    from concourse import mybir

    f32 = mybir.dt.float32
    f32r = mybir.dt.float32r
    Exp = mybir.ActivationFunctionType.Exp

    nc = bacc.Bacc("TRN2", target_bir_lowering=False, debug=False, num_devices=8)

    qT_in = nc.dram_tensor("qT_in", [D, S], f32, kind="ExternalInput")
    vT_in = nc.dram_tensor("vT_in", [D, S], f32, kind="ExternalInput")
    wq_d = nc.dram_tensor("wq", [D, 512], f32, kind="ExternalInput")
    wk_d = nc.dram_tensor("wk", [D, 512], f32, kind="ExternalInput")
    wv_d = nc.dram_tensor("wv", [D, 512], f32, kind="ExternalInput")
    wo_d = nc.dram_tensor("wo", [H * DH, 512], f32, kind="ExternalInput")
    bq_d = nc.dram_tensor("bq2", [P, 4], f32, kind="ExternalInput")
    bk_d = nc.dram_tensor("bk2", [P, 4], f32, kind="ExternalInput")
    bv_d = nc.dram_tensor("bv_row", [1, 512], f32, kind="ExternalInput")
    bo_d = nc.dram_tensor("bo_row", [1, 512], f32, kind="ExternalInput")
    vb_d = nc.dram_tensor("vbias", [P, 8], f32, kind="ExternalInput")
    qm_d = nc.dram_tensor("qm_rsh", [P, 16], f32, kind="ExternalInput")
    y_out = nc.dram_tensor("y_out", [S, 512], f32, kind="ExternalOutput")

    groups = [[0, 1], [2, 3], [4, 5], [6, 7]]

    def bcast_ap(src_ap, nparts):
        # partition-broadcast read (stride-0 partition dim); DRAM source only
        return bass.AP(
            tensor=src_ap.tensor,
            offset=src_ap.offset,
            ap=[[0, nparts]] + list(src_ap.ap[1:]),
        )

    with tile.TileContext(nc) as tc:
        with (
            tc.tile_pool(name="A", bufs=25) as A,
            tc.tile_pool(name="STBC", bufs=2) as STBC,
            tc.tile_pool(name="W", bufs=26) as Wp,
            tc.tile_pool(name="VS", bufs=8) as VSp,
            tc.tile_pool(name="SM", bufs=1) as SM,
            tc.tile_pool(name="SM2", bufs=2) as SM2,
            tc.tile_pool(name="ps_sc", bufs=2, space="PSUM") as PSC,
            tc.tile_pool(name="ps_wk", bufs=2, space="PSUM") as PSW,
            tc.tile_pool(name="ps_cx", bufs=2, space="PSUM") as PSX,
            tc.tile_pool(name="dram", bufs=4, space="DRAM") as DR,
        ):
            # ---- small constants ----
            bq_sb = SM.tile([P, 4], f32, tag="bq")
            nc.sync.dma_start(out=bq_sb[:], in_=bq_d[:, :])
            bk_sb = SM.tile([P, 4], f32, tag="bk")
            nc.sync.dma_start(out=bk_sb[:], in_=bk_d[:, :])
            vb_sb = SM.tile([P, 8], f32, tag="vb")
            nc.sync.dma_start(out=vb_sb[:], in_=vb_d[:, :])
            qm_sb = SM.tile([P, 16], f32, tag="qm")
            nc.sync.dma_start(out=qm_sb[:], in_=qm_d[:, :])
            bv_bc = SM.tile([P, 512], f32, tag="bvb")
            nc.gpsimd.dma_start(out=bv_bc[:], in_=bcast_ap(bv_d[:, :], P))
            bo_bc = SM.tile([P, 512], f32, tag="bob")
            nc.gpsimd.dma_start(out=bo_bc[:], in_=bcast_ap(bo_d[:, :], P))

            # ---- big loads (q/v pre-transposed on host) ----
            def load_xt(x_dram):
                out = []
                for i in range(8):
                    t = A.tile([P, S], f32, tag="big")
                    nc.sync.dma_start(
                        out=t[:].bitcast(f32r),
                        in_=x_dram[i * P:(i + 1) * P, :].bitcast(f32r),
                    )
                    out.append(t)
                return out

            def load_w(w_dram):
                out = []
                for i in range(8):
                    t = Wp.tile([P, 512], f32, tag="w")
                    nc.sync.dma_start(
                        out=t[:].bitcast(f32r),
                        in_=w_dram[i * P:(i + 1) * P, :].bitcast(f32r),
                    )
                    out.append(t)
                return out

            qT = load_xt(qT_in)
            wq_sb = load_w(wq_d)
            vT = load_xt(vT_in)
            wk_sb = load_w(wk_d)
            wv_sb = load_w(wv_d)

            QTp = [None] * 8   # per-head zero-padded Q^T [128, S]
            KT = [None] * 4    # stacked K^T head pairs [128, S]
            Vst = [None] * 8   # V with ones column [128, 8, 65]

            def q_group(ht):
                # Q^T for head pair ht -> two zero-padded per-head tiles
                tA = A.tile([P, S], f32, tag="big")
                nc.vector.memset(tA[64:128, :], 0.0)
                tB = A.tile([P, S], f32, tag="big")
                nc.vector.memset(tB[0:64, :], 0.0)
                for c in range(2):
                    cs = slice(c * 512, (c + 1) * 512)
                    ps = PSW.tile([P, 512], f32, tag="work")
                    for di in range(8):
                        nc.tensor.matmul(
                            ps[:, :],
                            lhsT=wq_sb[di][:, ht * P:(ht + 1) * P].bitcast(f32r),
                            rhs=qT[di][:, cs].bitcast(f32r),
                            start=(di == 0),
                            stop=(di == 7),
                        )
                    nc.vector.tensor_scalar_add(
                        tA[0:64, cs].bitcast(f32r), ps[0:64, :], bq_sb[0:64, ht:ht + 1]
                    )
                    nc.vector.tensor_scalar_add(
                        tB[64:128, cs].bitcast(f32r), ps[64:128, :], bq_sb[64:128, ht:ht + 1]
                    )
                QTp[2 * ht], QTp[2 * ht + 1] = tA, tB

            def k_group(ht):
                t = A.tile([P, S], f32, tag="big")
                for c in range(2):
                    cs = slice(c * 512, (c + 1) * 512)
                    ps = PSW.tile([P, 512], f32, tag="work")
                    for di in range(8):
                        nc.tensor.matmul(
                            ps[:, :],
                            lhsT=wk_sb[di][:, ht * P:(ht + 1) * P].bitcast(f32r),
                            rhs=vT[di][:, cs].bitcast(f32r),
                            start=(di == 0),
                            stop=(di == 7),
                        )
                    nc.vector.tensor_scalar_add(
                        t[:, cs].bitcast(f32r), ps[:, :], bk_sb[:, ht:ht + 1]
                    )
                KT[ht] = t

            def v_group(kt):
                ps = PSW.tile([P, 512], f32, tag="work")
                for di in range(8):
                    nc.tensor.matmul(
                        ps[:, :],
                        lhsT=vT[di][:, kt * P:(kt + 1) * P].bitcast(f32r),
                        rhs=wv_sb[di][:, :].bitcast(f32r),
                        start=(di == 0),
                        stop=(di == 7),
                    )
                t = VSp.tile([P, 8, 65], f32, tag="vst")
                nc.vector.memset(t[:], 1.0)
                nc.vector.tensor_add(
                    t[:, :, 0:64].bitcast(f32r),
                    ps[:, :].rearrange("p (h d) -> p h d", h=8),
                    bv_bc[:, :].rearrange("p (h d) -> p h d", h=8),
                )
                Vst[kt] = t

            ctxT_full = [None] * 8

            pair_state = {}

            def pair_compute(p):
                st = STBC.tile([P, S], f32, tag="st")
                sumA = SM2.tile([1, S], f32, tag="sumA")
                sumB = SM2.tile([1, S], f32, tag="sumB")
                for c in range(2):
                    cs = slice(c * 512, (c + 1) * 512)
                    ctxA = PSX.tile([65, 512], f32, tag="ctx")
                    ctxB = PSX.tile([65, 512], f32, tag="ctx")
                    for kt in range(8):
                        sps = PSC.tile([P, S], f32, tag="sc")
                        nc.tensor.matmul(
                            sps[:, 0:512],
                            lhsT=KT[p][:, kt * P:(kt + 1) * P].bitcast(f32r),
                            rhs=QTp[2 * p][:, cs].bitcast(f32r),
                            start=True,
                            stop=True,
                        )
                        nc.tensor.matmul(
                            sps[:, 512:1024],
                            lhsT=KT[p][:, kt * P:(kt + 1) * P].bitcast(f32r),
                            rhs=QTp[2 * p + 1][:, cs].bitcast(f32r),
                            start=True,
                            stop=True,
                        )
                        ut = A.tile([P, S], f32, tag="big")
                        nc.scalar.activation(
                            ut[:].bitcast(f32r), sps[:], Exp,
                            bias=vb_sb[:, kt:kt + 1], scale=1.0,
                        )
                        nc.tensor.matmul(
                            ctxA[:, :],
                            lhsT=Vst[kt][:, 2 * p, :].bitcast(f32r),
                            rhs=ut[:, 0:512].bitcast(f32r),
                            start=(kt == 0),
                            stop=(kt == 7),
                        )
                        nc.tensor.matmul(
                            ctxB[:, :],
                            lhsT=Vst[kt][:, 2 * p + 1, :].bitcast(f32r),
                            rhs=ut[:, 512:1024].bitcast(f32r),
                            start=(kt == 0),
                            stop=(kt == 7),
                        )
                    # evict ctx + sums out of PSUM right away (frees psum for
                    # the next chunk/pair); normalization happens on the copy
                    nc.vector.tensor_copy(st[0:64, cs], ctxA[0:64, :])
                    nc.vector.tensor_copy(sumA[0:1, cs], ctxA[64:65, :])
                    nc.vector.tensor_copy(st[64:128, cs], ctxB[0:64, :])
                    nc.vector.tensor_copy(sumB[0:1, cs], ctxB[64:65, :])
                pair_state[p] = (st, sumA, sumB)

            def pair_finish(p):
                st, sumA, sumB = pair_state[p]
                # r = q_mask / sums, broadcast over partitions via DRAM
                rsh = SM2.tile([P, 16], f32, tag="rsh")
                nc.gpsimd.dma_start(out=rsh[0:64, :], in_=sumA[:])
                nc.gpsimd.dma_start(out=rsh[64:128, :], in_=sumB[:])
                rr = SM2.tile([P, 16], f32, tag="rr")
                nc.vector.reciprocal(rr[:], rsh[:])
                nc.vector.tensor_mul(rr[:], rr[:], qm_sb[:])
                rdram = DR.tile([2, S], f32, tag="rd")
                nc.gpsimd.dma_start(out=rdram[0:1, :], in_=rr[0:64, :])
                nc.gpsimd.dma_start(out=rdram[1:2, :], in_=rr[64:128, :])
                bc = STBC.tile([P, S], f32, tag="bc")
                nc.gpsimd.dma_start(out=bc[0:64, :], in_=bcast_ap(rdram[0:1, :], 64))
                nc.gpsimd.dma_start(out=bc[64:128, :], in_=bcast_ap(rdram[1:2, :], 64))
                nc.vector.tensor_mul(st[:].bitcast(f32r), st[:], bc[:])

                # pairwise exchange of ctx^T
                cin = DR.tile([P, S], f32, tag="ccin")
                nc.gpsimd.dma_start(out=cin[:], in_=st[:])
                cout = DR.tile([2, P, S], f32, tag="ccout")
                nc.gpsimd.collective_compute(
                    "AllGather",
                    mybir.AluOpType.bypass,
                    replica_groups=groups,
                    ins=[cin[:].opt()],
                    outs=[cout[:].opt()],
                )
                ta = A.tile([P, S], f32, tag="big")
                nc.gpsimd.dma_start(out=ta[:].bitcast(f32r), in_=cout[0, :, :].bitcast(f32r))
                tb = A.tile([P, S], f32, tag="big")
                nc.gpsimd.dma_start(out=tb[:].bitcast(f32r), in_=cout[1, :, :].bitcast(f32r))
                ctxT_full[p] = ta
                ctxT_full[4 + p] = tb

            # ---- emit: projections upfront (frees qT/vT early), then pairs
            q_group(0)
            k_group(0)
            q_group(1)
            k_group(1)
            q_group(2)
            k_group(2)
            q_group(3)
            k_group(3)
            for kt in range(8):
                v_group(kt)
            wo_sb = load_w(wo_d)
            pair_compute(0)
            pair_compute(1)
            pair_finish(0)
            pair_compute(2)
            pair_finish(1)
            pair_compute(3)
            pair_finish(2)
            pair_finish(3)

            # ---- output projection (column slice), gather-arrival order ----
            HT_ORDER = [0, 4, 1, 5, 2, 6, 3, 7]
            for qt in range(8):
                yp = PSW.tile([P, 512], f32, tag="work")
                for i, ht in enumerate(HT_ORDER):
                    nc.tensor.matmul(
                        yp[:, 0:512],
                        lhsT=ctxT_full[ht][:, qt * P:(qt + 1) * P].bitcast(f32r),
                        rhs=wo_sb[ht][:, :].bitcast(f32r),
                        start=(i == 0),
                        stop=(i == 7),
                    )
                ysb = Wp.tile([P, 512], f32, tag="w")
                nc.vector.tensor_add(ysb[:], yp[:, 0:512], bo_bc[:])
                nc.sync.dma_start(out=y_out[qt * P:(qt + 1) * P, :], in_=ysb[:])

    nc.compile()
    return nc


def _get_program():
    if "nc" not in _CACHE:
        _CACHE["nc"] = _build_program()
    return _CACHE["nc"]


def kernel(q, v, q_mask, v_mask, Wq, bq, Wk, bk, Wv, bv, Wo, bo):
    global LAST_RESULT
    from concourse.bass_utils import run_bass_kernel_spmd

    q = np.asarray(q, dtype=np.float32)
    v = np.asarray(v, dtype=np.float32)
    q_mask = np.asarray(q_mask)
    v_mask = np.asarray(v_mask)
    Wq = np.asarray(Wq, dtype=np.float32)
    Wk = np.asarray(Wk, dtype=np.float32)
    Wv = np.asarray(Wv, dtype=np.float32)
    Wo = np.asarray(Wo, dtype=np.float32)
    bq = np.asarray(bq, dtype=np.float32)
    bk = np.asarray(bk, dtype=np.float32)
    bv = np.asarray(bv, dtype=np.float32)
    bo = np.asarray(bo, dtype=np.float32)

    nc = _get_program()

    in_maps = []
    for c in range(8):
        b, hh = c // 2, c % 2
        hsl = slice(512 * hh, 512 * (hh + 1))
        vb = np.where(v_mask[b], 0.0, NEG).astype(np.float32)
        qm = q_mask[b].astype(np.float32)
        in_maps.append(
            {
                "qT_in": np.ascontiguousarray(q[b].T),
                "vT_in": np.ascontiguousarray(v[b].T),
                "wq": np.ascontiguousarray(Wq[:, hsl]),
                "wk": np.ascontiguousarray(Wk[:, hsl]),
                "wv": np.ascontiguousarray(Wv[:, hsl]),
                "wo": np.ascontiguousarray(Wo[:, hsl]),
                "bq2": np.ascontiguousarray(bq[hsl].reshape(4, P).T),
                "bk2": np.ascontiguousarray(bk[hsl].reshape(4, P).T),
                "bv_row": np.ascontiguousarray(bv[hsl].reshape(1, 512)),
                "bo_row": np.ascontiguousarray(bo[hsl].reshape(1, 512)),
                "vbias": np.ascontiguousarray(vb.reshape(8, P).T),
                "qm_rsh": np.ascontiguousarray(
                    np.tile(qm.reshape(64, 16), (2, 1))
                ),
            }
        )

    res = run_bass_kernel_spmd(
        nc,
        in_maps,
        core_ids=list(range(8)),
        tmpdir=os.environ.get("KERNEL_TRACE_DIR") or None,
    )
    LAST_RESULT = res

    out = np.empty((B, S, D), dtype=np.float32)
    for b in range(B):
        out[b, :, 0:512] = res.results[2 * b]["y_out"]
        out[b, :, 512:1024] = res.results[2 * b + 1]["y_out"]
    return out
